# revision 22
# baseline (speedup 1.0000x reference)
"""Trainium2 kernel for nn_ContrastiveLoss (N=4096, D=1024), SPMD over 8 NeuronCores.

Strategy (row-sharded similarity matrix, fp8 DoubleRow matmuls):
  - Host (f64, O(N*D)): l2-normalize the four feature tensors, diag sims,
    pre_cos alignment term; scale back_* by 16 and quantize to e4m3 in
    DoubleRow-blocked layouts.
  - Each core (the O(N^2*D) part): its [512, 4096] slab of E = exp(Vn@An^T):
      * TensorE: dense HAM warmup (the PE clock gate opens after ~4us of
        gap-free activity; gaps >~3.4us reset the ramp), then 128 fp8
        DoubleRow matmuls (K=256, FD=512) at the full 2.4GHz rate (216ns
        issue-to-issue, 157 TF/s), then 8 bf16 ones-matmul partition folds
        for column sums. Folds run strictly AFTER the stream: a fold in the
        middle of the PE queue head-of-line blocks it on the Vector add
        chain (measured 6us stall + HAM gate close).
      * ScalarE: exp(PSUM/256) -> bf16 with fused f32 row-sum accumulator;
        exp-only during the stream.
      * VectorE: bf16 column-sum adds (2x DVE rate) + fold-PSUM copies.
      * DMA: one dma_start per 512KB an block (4KB/partition packets stripe
        across all 16 SDMA engines, ~185GB/s per HWDGE ring, both rings
        concurrently ~HBM cap); vn + even blocks on sync, odd on scalar.
    Outputs per core: rowsum chunks [128, 16], partial colsum [1, 4096].
  - Host: O(N) final assembly (log/ratio/sums) in f64.
"""

import os
import sys

import numpy as np

for _p in ("/opt/trn_rl_repo",):
    if _p not in sys.path and os.path.isdir(_p):
        sys.path.insert(0, _p)

N = 4096
D = 1024
NCORES = 8
ROWS = N // NCORES       # 512 rows per core
MCH = ROWS // 128        # 4 row chunks per core
KCH = D // 128           # 8 contraction chunks
NB = 512                 # matmul moving free dim
NCH = N // NB            # 8 column blocks

MARGIN = 0.2
BALANCE = 0.5
BIAS = 1.0
EPS = 1e-18

KD2 = KCH // 2   # fp8 DoubleRow: contraction chunks of 256 (2 x 128 rows)
FP8_SCALE = 16.0  # host pre-scale so e4m3 keeps the values out of subnormals

WARM_MM = 9      # dense HAM-warmup matmuls bridging the initial DMA wait

_CACHE = {}
LAST_RESULT = None  # BassKernelResults of the most recent run (for test harness)


def _build_nc():
    import concourse.bass as bass  # noqa: F401
    import concourse.bacc as bacc
    import concourse.tile as tile
    from concourse import mybir
    from contextlib import ExitStack

    BF16 = mybir.dt.bfloat16
    F32 = mybir.dt.float32
    Exp = mybir.ActivationFunctionType.Exp

    NP2 = NCH // 2  # column-block pairs; each exp covers 1024 cols

    nc = bacc.Bacc("TRN2", debug=False, num_devices=NCORES)

    FP8 = mybir.dt.float8e4
    DoubleRow = mybir.MatmulPerfMode.DoubleRow

    # DRAM I/O (per core). Layouts chosen so every DMA is one contiguous
    # [128, X] block (4KB per-partition packets -> full SDMA striping).
    # m-major so the m=0 weight chunk can land first in its own small DMA:
    # vnT[p, m*KCH*128 + k2*256 + i*128 + r] = Vn_slab[m*128+r, (2*k2+i)*128+p]
    vnT_d = nc.dram_tensor("vnT", [128, KCH * ROWS], FP8, kind="ExternalInput")
    # anT[n, p, k2*2*NB + i*NB + c] = An[n*NB + c, (2*k2+i)*128 + p] * FP8_SCALE
    anT_d = nc.dram_tensor("anT", [NCH, 128, KCH * NB], FP8, kind="ExternalInput")

    # rowsum[p, np2*MCH + m] = sum over cols [np2*1024,(np2+1)*1024) of
    #   E_slab[m*128 + p, :]; the last group's exp is split in halves with
    #   separate accumulators (cols 15,16) to shorten the tail chain
    rowsum_d = nc.dram_tensor("rowsum", [128, NP2 * MCH + 1], F32, kind="ExternalOutput")
    # colsum[0, j] = sum over this core's 512 rows of E[:, j]
    colsum_d = nc.dram_tensor("colsum", [1, N], F32, kind="ExternalOutput")

    with tile.TileContext(nc) as tc:
        with ExitStack() as ctx:
            singles = ctx.enter_context(tc.tile_pool(name="singles", bufs=1))

            ones_b = singles.tile([128, 1], BF16, tag="ones_b")
            dummy = singles.tile([128, NB], BF16, tag="dummy")
            nc.gpsimd.memset(ones_b[:], 1.0)
            nc.gpsimd.memset(dummy[:], 0.0)

            psum = ctx.enter_context(tc.tile_pool(name="mm_psum", bufs=3, space="PSUM"))
            foldp = ctx.enter_context(tc.tile_pool(name="fold_psum", bufs=2, space="PSUM"))
            epool = ctx.enter_context(tc.tile_pool(name="etile", bufs=3))

            # HAM warmup: dense matmul stream from t~=preamble-end so the PE
            # clock gate is at 8/8 right when the real stream's data lands.
            wps = foldp.tile([128, NB], mybir.dt.float32, tag="fold")
            for i in range(WARM_MM):
                nc.tensor.matmul(
                    wps[0:1, :], ones_b[:], dummy[:],
                    start=(i == 0), stop=(i == WARM_MM - 1),
                )

            # Input DMAs: one dma_start per block. Two HWDGE rings (sync,
            # scalar) drain FIFO, concurrently. Order matches consumption:
            # the m=0 weight chunk (128KB) + an0 head the sync ring so the
            # lo-half of the first group can start ~12us; an1 heads scalar's.
            vn_sb = singles.tile([128, KCH * ROWS], FP8, tag="vn")
            an_sb = []
            for n in range(NCH):
                an_t = singles.tile([128, KCH * NB], FP8, tag=f"an{n}")
                an_sb.append(an_t)
            VM = KCH * 128  # 1024 cols per m-chunk of vn
            AH = KD2 // 2 * 2 * NB  # an block k2-half: 2048 cols

            # First-needed pieces are small so completion-sem latency
            # (~1-1.7us after last byte) costs less on the critical path.
            nc.sync.dma_start(vn_sb[:, :VM], vnT_d.ap()[:, :VM])
            nc.scalar.dma_start(an_sb[1][:], anT_d.ap()[1])
            nc.sync.dma_start(an_sb[0][:, :AH], anT_d.ap()[0][:, :AH])
            nc.scalar.dma_start(an_sb[3][:], anT_d.ap()[3])
            nc.sync.dma_start(an_sb[0][:, AH:], anT_d.ap()[0][:, AH:])
            nc.scalar.dma_start(an_sb[5][:], anT_d.ap()[5])
            nc.sync.dma_start(vn_sb[:, VM : 2 * VM], vnT_d.ap()[:, VM : 2 * VM])
            nc.scalar.dma_start(an_sb[7][:], anT_d.ap()[7])
            nc.sync.dma_start(vn_sb[:, 2 * VM :], vnT_d.ap()[:, 2 * VM :])
            nc.sync.dma_start(an_sb[2][:], anT_d.ap()[2])
            nc.sync.dma_start(an_sb[4][:], anT_d.ap()[4])
            nc.sync.dma_start(an_sb[6][:], anT_d.ap()[6])

            efold16 = singles.tile([128, N], BF16, tag="efold16")
            rs = singles.tile([128, NP2 * MCH + 1], F32, tag="rs")
            colsb = singles.tile([1, N], F32, tag="colsb")

            # Column sums: partition-fold efold16 (bf16) with a ones-matmul,
            # then copy the [1, 512] PSUM row out.
            def fold(np2):
                for j in range(2):
                    nn = 2 * np2 + j
                    fps = foldp.tile([128, NB], mybir.dt.float32, tag="fold")
                    nc.tensor.matmul(
                        fps[0:1, :],
                        ones_b[:],
                        efold16[:, nn * NB : (nn + 1) * NB],
                        start=True,
                        stop=True,
                    )
                    if nn == NCH - 1:
                        # last copy on ScalarE (idle by now) so it runs in
                        # parallel with VectorE's copy of block NCH-2
                        nc.scalar.copy(colsb[:, nn * NB : (nn + 1) * NB], fps[0:1, :])
                    else:
                        nc.vector.tensor_scalar_add(
                            colsb[:, nn * NB : (nn + 1) * NB], fps[0:1, :], 0.0
                        )

            # Main similarity slab. Column-pair outer (np2), row-chunk inner:
            # each group accumulates 8 DoubleRow matmuls into a [128, 1024]
            # PSUM tile (2 banks), then one wide exp (bf16 out, fused f32
            # row-sum) drains it. Column sums accumulate in bf16 (2x DVE).
            descale = 1.0 / (FP8_SCALE * FP8_SCALE)
            for np2 in range(NP2):
                nlo, nhi = 2 * np2, 2 * np2 + 1
                for m in range(MCH):
                    ps = psum.tile([128, 2 * NB], mybir.dt.float32)
                    if np2 == 0 and m == 0:
                        # lo half first: these 4 matmuls need only vn_m0+an0,
                        # so the stream starts before an1 lands
                        order = [(0, nlo, k2) for k2 in range(KD2)] + [
                            (1, nhi, k2) for k2 in range(KD2)
                        ]
                    else:
                        order = [
                            (half, nn, k2)
                            for k2 in range(KD2)
                            for half, nn in ((0, nlo), (1, nhi))
                        ]
                    for half, nn, k2 in order:
                        w3 = (
                            vn_sb[:, m * VM + k2 * 256 : m * VM + (k2 + 1) * 256]
                            .rearrange("p (i c) -> p i c", i=2)
                        )
                        a3 = (
                            an_sb[nn][:, k2 * 2 * NB : (k2 + 1) * 2 * NB]
                            .rearrange("p (i c) -> p i c", i=2)
                        )
                        nc.tensor.matmul(
                            ps[:, half * NB : (half + 1) * NB],
                            w3,
                            a3,
                            start=(k2 == 0),
                            stop=(k2 == KD2 - 1),
                            perf_mode=DoubleRow,
                        )
                    col = np2 * MCH + m
                    sl = slice(np2 * 2 * NB, (np2 + 1) * 2 * NB)
                    if m == 0:
                        nc.scalar.activation(
                            efold16[:, sl], ps[:], Exp, scale=descale,
                            accum_out=rs[:, col : col + 1],
                        )
                    elif np2 == NP2 - 1 and m == MCH - 1:
                        # final group: exp in halves with separate accum
                        # columns; each half's exp can start as soon as its
                        # PSUM accumulation region closes, and its add/fold
                        # chain overlaps the other half's exp
                        et = epool.tile([128, 2 * NB], BF16)
                        for h in range(2):
                            nc.scalar.activation(
                                et[:, h * NB : (h + 1) * NB],
                                ps[:, h * NB : (h + 1) * NB], Exp, scale=descale,
                                accum_out=rs[:, col + h : col + h + 1],
                            )
                            hs = slice((np2 * 2 + h) * NB, (np2 * 2 + h + 1) * NB)
                            nc.vector.tensor_add(
                                efold16[:, hs], efold16[:, hs],
                                et[:, h * NB : (h + 1) * NB],
                            )
                    else:
                        et = epool.tile([128, 2 * NB], BF16)
                        nc.scalar.activation(
                            et[:], ps[:], Exp, scale=descale,
                            accum_out=rs[:, col : col + 1],
                        )
                        if m == MCH - 1:
                            # split in halves so each fold can start sooner
                            for h in range(2):
                                hs = slice(
                                    (np2 * 2 + h) * NB, (np2 * 2 + h + 1) * NB
                                )
                                nc.vector.tensor_add(
                                    efold16[:, hs], efold16[:, hs],
                                    et[:, h * NB : (h + 1) * NB],
                                )
                        else:
                            nc.vector.tensor_add(efold16[:, sl], efold16[:, sl], et[:])

            nc.scalar.dma_start(rowsum_d.ap(), rs[:])

            for np2 in range(NP2):
                fold(np2)
                if np2 == NP2 - 2:
                    # blocks 0..5 are final once fold(2) copies land; ship
                    # them while the last block's fold chain finishes
                    nc.sync.dma_start(
                        colsum_d.ap()[:, : 6 * NB], colsb[:, : 6 * NB]
                    )
            nc.sync.dma_start(colsum_d.ap()[:, 6 * NB :], colsb[:, 6 * NB :])

    nc.compile()
    return nc


def _get_nc():
    if "nc" not in _CACHE:
        _CACHE["nc"] = _build_nc()
    return _CACHE["nc"]


def _prep_inputs(pre_VF, pre_AF, back_VF, back_AF):
    """Normalize + relayout on host; returns per-core in_maps + host scalars."""
    import ml_dtypes

    def l2n(x):
        x = np.asarray(x, dtype=np.float64)
        return x / np.sqrt((x * x).sum(-1, keepdims=True) + EPS)

    Vn = l2n(back_VF)
    An = l2n(back_AF)
    diag = np.einsum("ij,ij->i", Vn, An)  # f64, exact-ish
    L_pre = float(np.einsum("ij,ij->i", l2n(pre_VF), l2n(pre_AF)).sum())

    fp8 = ml_dtypes.float8_e4m3
    Vn8 = (Vn * FP8_SCALE).astype(fp8)
    An8 = (An * FP8_SCALE).astype(fp8)

    # anT[n, p, k2*2*NB + i*NB + c] = An8[n*NB + c, (2*k2+i)*128 + p]
    anT = np.ascontiguousarray(
        An8.reshape(NCH, NB, KD2, 2, 128)
        .transpose(0, 4, 2, 3, 1)
        .reshape(NCH, 128, KCH * NB)
    )

    in_maps = []
    for c in range(NCORES):
        sl = slice(c * ROWS, (c + 1) * ROWS)
        # vnT[p, m*KCH*128 + k2*256 + i*128 + r] = Vn8_slab[m*128+r, (2k2+i)*128+p]
        vnT = np.ascontiguousarray(
            Vn8[sl]
            .reshape(MCH, 128, KD2, 2, 128)
            .transpose(4, 0, 2, 3, 1)
            .reshape(128, KCH * ROWS)
        )
        in_maps.append({"vnT": vnT, "anT": anT})
    return in_maps, diag, L_pre


def _assemble(outs, diag, L_pre):
    """O(N) final reduction on host, f64."""
    def _core_rowsum(c):
        r = outs[c]["rowsum"].astype(np.float64)  # [128, 17]
        r = np.concatenate([r[:, :15], (r[:, 15] + r[:, 16])[:, None]], axis=1)
        return r.reshape(128, NCH // 2, MCH).sum(1).T.reshape(ROWS)

    rowsum = np.concatenate([_core_rowsum(c) for c in range(NCORES)])
    colsum = np.zeros(N, dtype=np.float64)
    for c in range(NCORES):
        colsum += outs[c]["colsum"].astype(np.float64).reshape(N)

    dE = np.exp(diag)
    pos = np.exp(diag - MARGIN)
    neg_V = rowsum - dE
    neg_A = colsum - dE
    L_V = np.log(pos / (pos + neg_V)).sum()
    L_A = np.log(pos / (pos + neg_A)).sum()

    loss = BALANCE * (-1.0 / BIAS) * (L_V + L_A) + (1.0 - BALANCE) * L_pre
    return np.array(loss, dtype=np.float32)


def kernel(pre_VF, pre_AF, back_VF, back_AF):
    global LAST_RESULT
    from concourse import bass_utils

    nc = _get_nc()
    in_maps, diag, L_pre = _prep_inputs(pre_VF, pre_AF, back_VF, back_AF)
    res = bass_utils.run_bass_kernel_spmd(nc, in_maps, core_ids=list(range(NCORES)))
    LAST_RESULT = res
    return _assemble(res.results, diag, L_pre)


# revision 23
# speedup vs baseline: 1.0339x; 1.0339x over previous
"""Trainium2 kernel for nn_ContrastiveLoss (N=4096, D=1024), SPMD over 8 NeuronCores.

Strategy (row-sharded similarity matrix, fp8 DoubleRow matmuls):
  - Host (f64, O(N*D)): l2-normalize the four feature tensors, diag sims,
    pre_cos alignment term; scale back_* by 16 and quantize to e4m3 in
    DoubleRow-blocked layouts.
  - Each core (the O(N^2*D) part): its [512, 4096] slab of E = exp(Vn@An^T):
      * TensorE: dense HAM warmup (the PE clock gate opens after ~4us of
        gap-free activity; gaps >~3.4us reset the ramp), then 128 fp8
        DoubleRow matmuls (K=256, FD=512) at the full 2.4GHz rate (216ns
        issue-to-issue, 157 TF/s), then 8 bf16 ones-matmul partition folds
        for column sums. Folds run strictly AFTER the stream: a fold in the
        middle of the PE queue head-of-line blocks it on the Vector add
        chain (measured 6us stall + HAM gate close).
      * ScalarE: exp(PSUM/256) -> bf16 with fused f32 row-sum accumulator;
        exp-only during the stream.
      * VectorE: bf16 column-sum adds (2x DVE rate) + fold-PSUM copies.
      * DMA: one dma_start per 512KB an block (4KB/partition packets stripe
        across all 16 SDMA engines, ~185GB/s per HWDGE ring, both rings
        concurrently ~HBM cap); vn + even blocks on sync, odd on scalar.
    Outputs per core: rowsum chunks [128, 16], partial colsum [1, 4096].
  - Host: O(N) final assembly (log/ratio/sums) in f64.
"""

import os
import sys

import numpy as np

for _p in ("/opt/trn_rl_repo",):
    if _p not in sys.path and os.path.isdir(_p):
        sys.path.insert(0, _p)

N = 4096
D = 1024
NCORES = 8
ROWS = N // NCORES       # 512 rows per core
MCH = ROWS // 128        # 4 row chunks per core
KCH = D // 128           # 8 contraction chunks
NB = 512                 # matmul moving free dim
NCH = N // NB            # 8 column blocks

MARGIN = 0.2
BALANCE = 0.5
BIAS = 1.0
EPS = 1e-18

KD2 = KCH // 2   # fp8 DoubleRow: contraction chunks of 256 (2 x 128 rows)
FP8_SCALE = 16.0  # host pre-scale so e4m3 keeps the values out of subnormals

# Dense HAM-warmup matmuls bridging the initial DMA wait. Sized for the
# WORST-CASE first-block arrival (~13us; DMA ring spin-up jitters +-1.5us):
# a PE gap >~1.5-3us closes the HAM clock gate and costs a ~6us re-ramp at
# half duty, while each extra warmup matmul costs only ~0.43us of overlap.
WARM_MM = 12

_CACHE = {}
LAST_RESULT = None  # BassKernelResults of the most recent run (for test harness)


def _build_nc():
    import concourse.bass as bass  # noqa: F401
    import concourse.bacc as bacc
    import concourse.tile as tile
    from concourse import mybir
    from contextlib import ExitStack

    BF16 = mybir.dt.bfloat16
    F32 = mybir.dt.float32
    Exp = mybir.ActivationFunctionType.Exp

    NP2 = NCH // 2  # column-block pairs; each exp covers 1024 cols

    nc = bacc.Bacc("TRN2", debug=False, num_devices=NCORES)

    FP8 = mybir.dt.float8e4
    DoubleRow = mybir.MatmulPerfMode.DoubleRow

    # DRAM I/O (per core). Layouts chosen so every DMA is one contiguous
    # [128, X] block (4KB per-partition packets -> full SDMA striping).
    # m-major so the m=0 weight chunk can land first in its own small DMA:
    # vnT[p, m*KCH*128 + k2*256 + i*128 + r] = Vn_slab[m*128+r, (2*k2+i)*128+p]
    vnT_d = nc.dram_tensor("vnT", [128, KCH * ROWS], FP8, kind="ExternalInput")
    # anT[n, p, k2*2*NB + i*NB + c] = An[n*NB + c, (2*k2+i)*128 + p] * FP8_SCALE
    anT_d = nc.dram_tensor("anT", [NCH, 128, KCH * NB], FP8, kind="ExternalInput")

    # rowsum[p, np2*MCH + m] = sum over cols [np2*1024,(np2+1)*1024) of
    #   E_slab[m*128 + p, :]; the last group's exp is split in halves with
    #   separate accumulators (cols 15,16) to shorten the tail chain
    rowsum_d = nc.dram_tensor("rowsum", [128, NP2 * MCH + 1], F32, kind="ExternalOutput")
    # colsum[0, j] = sum over this core's 512 rows of E[:, j]
    colsum_d = nc.dram_tensor("colsum", [1, N], F32, kind="ExternalOutput")

    with tile.TileContext(nc) as tc:
        with ExitStack() as ctx:
            singles = ctx.enter_context(tc.tile_pool(name="singles", bufs=1))

            ones_b = singles.tile([128, 1], BF16, tag="ones_b")
            dummy = singles.tile([128, NB], BF16, tag="dummy")
            nc.gpsimd.memset(ones_b[:], 1.0)
            nc.gpsimd.memset(dummy[:], 0.0)

            psum = ctx.enter_context(tc.tile_pool(name="mm_psum", bufs=3, space="PSUM"))
            foldp = ctx.enter_context(tc.tile_pool(name="fold_psum", bufs=2, space="PSUM"))
            epool = ctx.enter_context(tc.tile_pool(name="etile", bufs=3))

            # HAM warmup: dense matmul stream from t~=preamble-end so the PE
            # clock gate is at 8/8 right when the real stream's data lands.
            wps = foldp.tile([128, NB], mybir.dt.float32, tag="fold")
            for i in range(WARM_MM):
                nc.tensor.matmul(
                    wps[0:1, :], ones_b[:], dummy[:],
                    start=(i == 0), stop=(i == WARM_MM - 1),
                )

            # Input DMAs: one dma_start per block. Two HWDGE rings (sync,
            # scalar) drain FIFO, concurrently. Order matches consumption:
            # the m=0 weight chunk (128KB) + an0 head the sync ring so the
            # lo-half of the first group can start ~12us; an1 heads scalar's.
            vn_sb = singles.tile([128, KCH * ROWS], FP8, tag="vn")
            an_sb = []
            for n in range(NCH):
                an_t = singles.tile([128, KCH * NB], FP8, tag=f"an{n}")
                an_sb.append(an_t)
            VM = KCH * 128  # 1024 cols per m-chunk of vn
            AH = KD2 // 2 * 2 * NB  # an block k2-half: 2048 cols

            # First-needed pieces are small so completion-sem latency
            # (~1-1.7us after last byte) costs less on the critical path.
            nc.sync.dma_start(vn_sb[:, :VM], vnT_d.ap()[:, :VM])
            nc.scalar.dma_start(an_sb[1][:], anT_d.ap()[1])
            nc.sync.dma_start(an_sb[0][:, :AH], anT_d.ap()[0][:, :AH])
            nc.scalar.dma_start(an_sb[3][:], anT_d.ap()[3])
            nc.sync.dma_start(an_sb[0][:, AH:], anT_d.ap()[0][:, AH:])
            nc.scalar.dma_start(an_sb[5][:], anT_d.ap()[5])
            nc.sync.dma_start(vn_sb[:, VM : 2 * VM], vnT_d.ap()[:, VM : 2 * VM])
            nc.scalar.dma_start(an_sb[7][:], anT_d.ap()[7])
            nc.sync.dma_start(vn_sb[:, 2 * VM :], vnT_d.ap()[:, 2 * VM :])
            nc.sync.dma_start(an_sb[2][:], anT_d.ap()[2])
            nc.sync.dma_start(an_sb[4][:], anT_d.ap()[4])
            nc.sync.dma_start(an_sb[6][:], anT_d.ap()[6])

            efold16 = singles.tile([128, N], BF16, tag="efold16")
            rs = singles.tile([128, NP2 * MCH + 1], F32, tag="rs")
            colsb = singles.tile([1, N], F32, tag="colsb")

            # Column sums: partition-fold efold16 (bf16) with a ones-matmul,
            # then copy the [1, 512] PSUM row out.
            def fold(np2):
                for j in range(2):
                    nn = 2 * np2 + j
                    fps = foldp.tile([128, NB], mybir.dt.float32, tag="fold")
                    nc.tensor.matmul(
                        fps[0:1, :],
                        ones_b[:],
                        efold16[:, nn * NB : (nn + 1) * NB],
                        start=True,
                        stop=True,
                    )
                    if nn == NCH - 1:
                        # last copy on ScalarE (idle by now) so it runs in
                        # parallel with VectorE's copy of block NCH-2
                        nc.scalar.copy(colsb[:, nn * NB : (nn + 1) * NB], fps[0:1, :])
                    else:
                        nc.vector.tensor_scalar_add(
                            colsb[:, nn * NB : (nn + 1) * NB], fps[0:1, :], 0.0
                        )

            # Main similarity slab. Column-pair outer (np2), row-chunk inner:
            # each group accumulates 8 DoubleRow matmuls into a [128, 1024]
            # PSUM tile (2 banks), then one wide exp (bf16 out, fused f32
            # row-sum) drains it. Column sums accumulate in bf16 (2x DVE).
            descale = 1.0 / (FP8_SCALE * FP8_SCALE)
            for np2 in range(NP2):
                nlo, nhi = 2 * np2, 2 * np2 + 1
                for m in range(MCH):
                    ps = psum.tile([128, 2 * NB], mybir.dt.float32)
                    if np2 == 0 and m == 0:
                        # lo half first: these 4 matmuls need only vn_m0+an0,
                        # so the stream starts before an1 lands
                        order = [(0, nlo, k2) for k2 in range(KD2)] + [
                            (1, nhi, k2) for k2 in range(KD2)
                        ]
                    else:
                        order = [
                            (half, nn, k2)
                            for k2 in range(KD2)
                            for half, nn in ((0, nlo), (1, nhi))
                        ]
                    for half, nn, k2 in order:
                        w3 = (
                            vn_sb[:, m * VM + k2 * 256 : m * VM + (k2 + 1) * 256]
                            .rearrange("p (i c) -> p i c", i=2)
                        )
                        a3 = (
                            an_sb[nn][:, k2 * 2 * NB : (k2 + 1) * 2 * NB]
                            .rearrange("p (i c) -> p i c", i=2)
                        )
                        nc.tensor.matmul(
                            ps[:, half * NB : (half + 1) * NB],
                            w3,
                            a3,
                            start=(k2 == 0),
                            stop=(k2 == KD2 - 1),
                            perf_mode=DoubleRow,
                        )
                    col = np2 * MCH + m
                    sl = slice(np2 * 2 * NB, (np2 + 1) * 2 * NB)
                    if m == 0:
                        nc.scalar.activation(
                            efold16[:, sl], ps[:], Exp, scale=descale,
                            accum_out=rs[:, col : col + 1],
                        )
                    elif np2 == NP2 - 1 and m == MCH - 1:
                        # final group: exp in halves with separate accum
                        # columns; each half's exp can start as soon as its
                        # PSUM accumulation region closes, and its add/fold
                        # chain overlaps the other half's exp
                        et = epool.tile([128, 2 * NB], BF16)
                        for h in range(2):
                            nc.scalar.activation(
                                et[:, h * NB : (h + 1) * NB],
                                ps[:, h * NB : (h + 1) * NB], Exp, scale=descale,
                                accum_out=rs[:, col + h : col + h + 1],
                            )
                            hs = slice((np2 * 2 + h) * NB, (np2 * 2 + h + 1) * NB)
                            nc.vector.tensor_add(
                                efold16[:, hs], efold16[:, hs],
                                et[:, h * NB : (h + 1) * NB],
                            )
                    else:
                        et = epool.tile([128, 2 * NB], BF16)
                        nc.scalar.activation(
                            et[:], ps[:], Exp, scale=descale,
                            accum_out=rs[:, col : col + 1],
                        )
                        if m == MCH - 1:
                            # split in halves so each fold can start sooner
                            for h in range(2):
                                hs = slice(
                                    (np2 * 2 + h) * NB, (np2 * 2 + h + 1) * NB
                                )
                                nc.vector.tensor_add(
                                    efold16[:, hs], efold16[:, hs],
                                    et[:, h * NB : (h + 1) * NB],
                                )
                        else:
                            nc.vector.tensor_add(efold16[:, sl], efold16[:, sl], et[:])

            nc.scalar.dma_start(rowsum_d.ap(), rs[:])

            for np2 in range(NP2):
                fold(np2)
                if np2 == NP2 - 2:
                    # blocks 0..5 are final once fold(2) copies land; ship
                    # them while the last block's fold chain finishes
                    nc.sync.dma_start(
                        colsum_d.ap()[:, : 6 * NB], colsb[:, : 6 * NB]
                    )
            nc.sync.dma_start(colsum_d.ap()[:, 6 * NB :], colsb[:, 6 * NB :])

    nc.compile()
    return nc


def _get_nc():
    if "nc" not in _CACHE:
        _CACHE["nc"] = _build_nc()
    return _CACHE["nc"]


def _prep_inputs(pre_VF, pre_AF, back_VF, back_AF):
    """Normalize + relayout on host; returns per-core in_maps + host scalars."""
    import ml_dtypes

    def l2n(x):
        x = np.asarray(x, dtype=np.float64)
        return x / np.sqrt((x * x).sum(-1, keepdims=True) + EPS)

    Vn = l2n(back_VF)
    An = l2n(back_AF)
    diag = np.einsum("ij,ij->i", Vn, An)  # f64, exact-ish
    L_pre = float(np.einsum("ij,ij->i", l2n(pre_VF), l2n(pre_AF)).sum())

    fp8 = ml_dtypes.float8_e4m3
    Vn8 = (Vn * FP8_SCALE).astype(fp8)
    An8 = (An * FP8_SCALE).astype(fp8)

    # anT[n, p, k2*2*NB + i*NB + c] = An8[n*NB + c, (2*k2+i)*128 + p]
    anT = np.ascontiguousarray(
        An8.reshape(NCH, NB, KD2, 2, 128)
        .transpose(0, 4, 2, 3, 1)
        .reshape(NCH, 128, KCH * NB)
    )

    in_maps = []
    for c in range(NCORES):
        sl = slice(c * ROWS, (c + 1) * ROWS)
        # vnT[p, m*KCH*128 + k2*256 + i*128 + r] = Vn8_slab[m*128+r, (2k2+i)*128+p]
        vnT = np.ascontiguousarray(
            Vn8[sl]
            .reshape(MCH, 128, KD2, 2, 128)
            .transpose(4, 0, 2, 3, 1)
            .reshape(128, KCH * ROWS)
        )
        in_maps.append({"vnT": vnT, "anT": anT})
    return in_maps, diag, L_pre


def _assemble(outs, diag, L_pre):
    """O(N) final reduction on host, f64."""
    def _core_rowsum(c):
        r = outs[c]["rowsum"].astype(np.float64)  # [128, 17]
        r = np.concatenate([r[:, :15], (r[:, 15] + r[:, 16])[:, None]], axis=1)
        return r.reshape(128, NCH // 2, MCH).sum(1).T.reshape(ROWS)

    rowsum = np.concatenate([_core_rowsum(c) for c in range(NCORES)])
    colsum = np.zeros(N, dtype=np.float64)
    for c in range(NCORES):
        colsum += outs[c]["colsum"].astype(np.float64).reshape(N)

    dE = np.exp(diag)
    pos = np.exp(diag - MARGIN)
    neg_V = rowsum - dE
    neg_A = colsum - dE
    L_V = np.log(pos / (pos + neg_V)).sum()
    L_A = np.log(pos / (pos + neg_A)).sum()

    loss = BALANCE * (-1.0 / BIAS) * (L_V + L_A) + (1.0 - BALANCE) * L_pre
    return np.array(loss, dtype=np.float32)


def kernel(pre_VF, pre_AF, back_VF, back_AF):
    global LAST_RESULT
    from concourse import bass_utils

    nc = _get_nc()
    in_maps, diag, L_pre = _prep_inputs(pre_VF, pre_AF, back_VF, back_AF)
    res = bass_utils.run_bass_kernel_spmd(nc, in_maps, core_ids=list(range(NCORES)))
    LAST_RESULT = res
    return _assemble(res.results, diag, L_pre)


# revision 25
# speedup vs baseline: 1.0739x; 1.0388x over previous
"""Trainium2 kernel for nn_ContrastiveLoss (N=4096, D=1024), SPMD over 8 NeuronCores.

Strategy (row-sharded similarity matrix, fp8 DoubleRow matmuls):
  - Host (f64, O(N*D)): l2-normalize the four feature tensors, diag sims,
    pre_cos alignment term; scale back_* by 16 and quantize to e4m3 in
    DoubleRow-blocked layouts.
  - Each core (the O(N^2*D) part): its [512, 4096] slab of E = exp(Vn@An^T):
      * TensorE: dense HAM warmup (the PE clock gate opens after ~4us of
        gap-free activity; gaps >~3.4us reset the ramp), then 128 fp8
        DoubleRow matmuls (K=256, FD=512) at the full 2.4GHz rate (216ns
        issue-to-issue, 157 TF/s), then 8 bf16 ones-matmul partition folds
        for column sums. Folds run strictly AFTER the stream: a fold in the
        middle of the PE queue head-of-line blocks it on the Vector add
        chain (measured 6us stall + HAM gate close).
      * ScalarE: exp(PSUM/256) -> bf16 with fused f32 row-sum accumulator;
        exp-only during the stream.
      * VectorE: bf16 column-sum adds (2x DVE rate) + fold-PSUM copies.
      * DMA: one dma_start per 512KB an block (4KB/partition packets stripe
        across all 16 SDMA engines, ~185GB/s per HWDGE ring, both rings
        concurrently ~HBM cap); vn + even blocks on sync, odd on scalar.
    Outputs per core: rowsum chunks [128, 16], partial colsum [1, 4096].
  - Host: O(N) final assembly (log/ratio/sums) in f64.
"""

import os
import sys

import numpy as np

for _p in ("/opt/trn_rl_repo",):
    if _p not in sys.path and os.path.isdir(_p):
        sys.path.insert(0, _p)

N = 4096
D = 1024
NCORES = 8
ROWS = N // NCORES       # 512 rows per core
MCH = ROWS // 128        # 4 row chunks per core
KCH = D // 128           # 8 contraction chunks
NB = 512                 # matmul moving free dim
NCH = N // NB            # 8 column blocks

MARGIN = 0.2
BALANCE = 0.5
BIAS = 1.0
EPS = 1e-18

KD2 = KCH // 2   # fp8 DoubleRow: contraction chunks of 256 (2 x 128 rows)
FP8_SCALE = 16.0  # host pre-scale so e4m3 keeps the values out of subnormals

# Dense HAM-warmup matmuls bridging the initial DMA wait. Sized for the
# WORST-CASE first-block arrival (~13us; DMA ring spin-up jitters +-1.5us):
# a PE gap >~1.5-3us closes the HAM clock gate and costs a ~6us re-ramp at
# half duty, while each extra warmup matmul costs only ~0.43us of overlap.
WARM_MM = 12

_CACHE = {}
LAST_RESULT = None  # BassKernelResults of the most recent run (for test harness)


def _build_nc():
    import concourse.bass as bass  # noqa: F401
    import concourse.bacc as bacc
    import concourse.tile as tile
    from concourse import mybir
    from contextlib import ExitStack

    BF16 = mybir.dt.bfloat16
    F32 = mybir.dt.float32
    Exp = mybir.ActivationFunctionType.Exp

    NP2 = NCH // 2  # column-block pairs; each exp covers 1024 cols

    nc = bacc.Bacc("TRN2", debug=False, num_devices=NCORES)

    FP8 = mybir.dt.float8e4
    DoubleRow = mybir.MatmulPerfMode.DoubleRow

    # DRAM I/O (per core). Layouts chosen so every DMA is one contiguous
    # [128, X] block (4KB per-partition packets -> full SDMA striping).
    # m-major so the m=0 weight chunk can land first in its own small DMA:
    # vnT[p, m*KCH*128 + k2*256 + i*128 + r] = Vn_slab[m*128+r, (2*k2+i)*128+p]
    vnT_d = nc.dram_tensor("vnT", [128, KCH * ROWS], FP8, kind="ExternalInput")
    # anT[n, p, k2*2*NB + i*NB + c] = An[n*NB + c, (2*k2+i)*128 + p] * FP8_SCALE
    anT_d = nc.dram_tensor("anT", [NCH, 128, KCH * NB], FP8, kind="ExternalInput")

    # rowsum[p, np2*MCH + m] = sum over cols [np2*1024,(np2+1)*1024) of
    #   E_slab[m*128 + p, :]; the last group's exp is split in halves with
    #   separate accumulators (cols 15,16) to shorten the tail chain
    rowsum_d = nc.dram_tensor("rowsum", [128, NP2 * MCH + 1], F32, kind="ExternalOutput")
    # colsum[0, j] = sum over this core's 512 rows of E[:, j]
    colsum_d = nc.dram_tensor("colsum", [1, N], F32, kind="ExternalOutput")

    with tile.TileContext(nc) as tc:
        with ExitStack() as ctx:
            singles = ctx.enter_context(tc.tile_pool(name="singles", bufs=1))

            ones_b = singles.tile([128, 1], BF16, tag="ones_b")
            dummy = singles.tile([128, NB], BF16, tag="dummy")
            nc.gpsimd.memset(ones_b[:], 1.0)
            nc.gpsimd.memset(dummy[:], 0.0)

            psum = ctx.enter_context(tc.tile_pool(name="mm_psum", bufs=3, space="PSUM"))
            foldp = ctx.enter_context(tc.tile_pool(name="fold_psum", bufs=2, space="PSUM"))
            epool = ctx.enter_context(tc.tile_pool(name="etile", bufs=3))

            # HAM warmup: dense matmul stream from t~=preamble-end so the PE
            # clock gate is at 8/8 right when the real stream's data lands.
            wps = foldp.tile([128, NB], mybir.dt.float32, tag="fold")
            for i in range(WARM_MM):
                nc.tensor.matmul(
                    wps[0:1, :], ones_b[:], dummy[:],
                    start=(i == 0), stop=(i == WARM_MM - 1),
                )

            # Input DMAs: one dma_start per block. Two HWDGE rings (sync,
            # scalar) drain FIFO, concurrently. Order matches consumption:
            # the m=0 weight chunk (128KB) + an0 head the sync ring so the
            # lo-half of the first group can start ~12us; an1 heads scalar's.
            vn_sb = singles.tile([128, KCH * ROWS], FP8, tag="vn")
            an_sb = []
            for n in range(NCH):
                an_t = singles.tile([128, KCH * NB], FP8, tag=f"an{n}")
                an_sb.append(an_t)
            VM = KCH * 128  # 1024 cols per m-chunk of vn
            AH = KD2 // 2 * 2 * NB  # an block k2-half: 2048 cols

            # First-needed pieces are small (completion-sem latency ~1us
            # after last byte) and STRIPED ACROSS BOTH RINGS: which ring
            # spins up first is random run-to-run (+-1.5us), so an0/an1 each
            # have one half per ring — either draw gives a gapless ~13us
            # stream start (a >~1.9us ramp-phase stall demotes the HAM gate
            # for a 3.4us duty window, costing ~6us).
            nc.sync.dma_start(vn_sb[:, :VM], vnT_d.ap()[:, :VM])
            nc.scalar.dma_start(an_sb[1][:, :AH], anT_d.ap()[1][:, :AH])
            nc.sync.dma_start(an_sb[0][:, :AH], anT_d.ap()[0][:, :AH])
            nc.scalar.dma_start(an_sb[0][:, AH:], anT_d.ap()[0][:, AH:])
            nc.sync.dma_start(an_sb[1][:, AH:], anT_d.ap()[1][:, AH:])
            nc.scalar.dma_start(vn_sb[:, 2 * VM :], vnT_d.ap()[:, 2 * VM :])
            nc.sync.dma_start(vn_sb[:, VM : 2 * VM], vnT_d.ap()[:, VM : 2 * VM])
            nc.scalar.dma_start(an_sb[3][:], anT_d.ap()[3])
            nc.sync.dma_start(an_sb[2][:], anT_d.ap()[2])
            nc.scalar.dma_start(an_sb[5][:], anT_d.ap()[5])
            nc.sync.dma_start(an_sb[4][:], anT_d.ap()[4])
            nc.scalar.dma_start(an_sb[7][:], anT_d.ap()[7])
            nc.sync.dma_start(an_sb[6][:], anT_d.ap()[6])

            efold16 = singles.tile([128, N], BF16, tag="efold16")
            rs = singles.tile([128, NP2 * MCH + 1], F32, tag="rs")
            colsb = singles.tile([1, N], F32, tag="colsb")

            # Column sums: partition-fold efold16 (bf16) with a ones-matmul,
            # then copy the [1, 512] PSUM row out.
            def fold(np2):
                for j in range(2):
                    nn = 2 * np2 + j
                    fps = foldp.tile([128, NB], mybir.dt.float32, tag="fold")
                    nc.tensor.matmul(
                        fps[0:1, :],
                        ones_b[:],
                        efold16[:, nn * NB : (nn + 1) * NB],
                        start=True,
                        stop=True,
                    )
                    if nn == NCH - 1:
                        # last copy on ScalarE (idle by now) so it runs in
                        # parallel with VectorE's copy of block NCH-2
                        nc.scalar.copy(colsb[:, nn * NB : (nn + 1) * NB], fps[0:1, :])
                    else:
                        nc.vector.tensor_scalar_add(
                            colsb[:, nn * NB : (nn + 1) * NB], fps[0:1, :], 0.0
                        )

            # Main similarity slab. Column-pair outer (np2), row-chunk inner:
            # each group accumulates 8 DoubleRow matmuls into a [128, 1024]
            # PSUM tile (2 banks), then one wide exp (bf16 out, fused f32
            # row-sum) drains it. Column sums accumulate in bf16 (2x DVE).
            descale = 1.0 / (FP8_SCALE * FP8_SCALE)
            for np2 in range(NP2):
                nlo, nhi = 2 * np2, 2 * np2 + 1
                for m in range(MCH):
                    ps = psum.tile([128, 2 * NB], mybir.dt.float32)
                    if np2 == 0 and m == 0:
                        # k2-halves first (each an half-block is one DMA
                        # piece): matches piece arrival under EITHER ring
                        # spin-up order
                        order = [
                            (0, nlo, 0), (0, nlo, 1), (1, nhi, 0), (1, nhi, 1),
                            (0, nlo, 2), (0, nlo, 3), (1, nhi, 2), (1, nhi, 3),
                        ]
                    else:
                        order = [
                            (half, nn, k2)
                            for k2 in range(KD2)
                            for half, nn in ((0, nlo), (1, nhi))
                        ]
                    for half, nn, k2 in order:
                        w3 = (
                            vn_sb[:, m * VM + k2 * 256 : m * VM + (k2 + 1) * 256]
                            .rearrange("p (i c) -> p i c", i=2)
                        )
                        a3 = (
                            an_sb[nn][:, k2 * 2 * NB : (k2 + 1) * 2 * NB]
                            .rearrange("p (i c) -> p i c", i=2)
                        )
                        nc.tensor.matmul(
                            ps[:, half * NB : (half + 1) * NB],
                            w3,
                            a3,
                            start=(k2 == 0),
                            stop=(k2 == KD2 - 1),
                            perf_mode=DoubleRow,
                        )
                    col = np2 * MCH + m
                    sl = slice(np2 * 2 * NB, (np2 + 1) * 2 * NB)
                    if m == 0:
                        nc.scalar.activation(
                            efold16[:, sl], ps[:], Exp, scale=descale,
                            accum_out=rs[:, col : col + 1],
                        )
                    elif np2 == NP2 - 1 and m == MCH - 1:
                        # final group: exp in halves with separate accum
                        # columns; each half's exp can start as soon as its
                        # PSUM accumulation region closes, and its add/fold
                        # chain overlaps the other half's exp
                        et = epool.tile([128, 2 * NB], BF16)
                        for h in range(2):
                            nc.scalar.activation(
                                et[:, h * NB : (h + 1) * NB],
                                ps[:, h * NB : (h + 1) * NB], Exp, scale=descale,
                                accum_out=rs[:, col + h : col + h + 1],
                            )
                            hs = slice((np2 * 2 + h) * NB, (np2 * 2 + h + 1) * NB)
                            nc.vector.tensor_add(
                                efold16[:, hs], efold16[:, hs],
                                et[:, h * NB : (h + 1) * NB],
                            )
                    else:
                        et = epool.tile([128, 2 * NB], BF16)
                        nc.scalar.activation(
                            et[:], ps[:], Exp, scale=descale,
                            accum_out=rs[:, col : col + 1],
                        )
                        if m == MCH - 1:
                            # split in halves so each fold can start sooner
                            for h in range(2):
                                hs = slice(
                                    (np2 * 2 + h) * NB, (np2 * 2 + h + 1) * NB
                                )
                                nc.vector.tensor_add(
                                    efold16[:, hs], efold16[:, hs],
                                    et[:, h * NB : (h + 1) * NB],
                                )
                        else:
                            nc.vector.tensor_add(efold16[:, sl], efold16[:, sl], et[:])

            nc.scalar.dma_start(rowsum_d.ap(), rs[:])

            for np2 in range(NP2):
                fold(np2)
                if np2 == NP2 - 2:
                    # blocks 0..5 are final once fold(2) copies land; ship
                    # them while the last block's fold chain finishes
                    nc.sync.dma_start(
                        colsum_d.ap()[:, : 6 * NB], colsb[:, : 6 * NB]
                    )
            nc.sync.dma_start(colsum_d.ap()[:, 6 * NB :], colsb[:, 6 * NB :])

    nc.compile()
    return nc


def _get_nc():
    if "nc" not in _CACHE:
        _CACHE["nc"] = _build_nc()
    return _CACHE["nc"]


def _prep_inputs(pre_VF, pre_AF, back_VF, back_AF):
    """Normalize + relayout on host; returns per-core in_maps + host scalars."""
    import ml_dtypes

    def l2n(x):
        x = np.asarray(x, dtype=np.float64)
        return x / np.sqrt((x * x).sum(-1, keepdims=True) + EPS)

    Vn = l2n(back_VF)
    An = l2n(back_AF)
    diag = np.einsum("ij,ij->i", Vn, An)  # f64, exact-ish
    L_pre = float(np.einsum("ij,ij->i", l2n(pre_VF), l2n(pre_AF)).sum())

    fp8 = ml_dtypes.float8_e4m3
    Vn8 = (Vn * FP8_SCALE).astype(fp8)
    An8 = (An * FP8_SCALE).astype(fp8)

    # anT[n, p, k2*2*NB + i*NB + c] = An8[n*NB + c, (2*k2+i)*128 + p]
    anT = np.ascontiguousarray(
        An8.reshape(NCH, NB, KD2, 2, 128)
        .transpose(0, 4, 2, 3, 1)
        .reshape(NCH, 128, KCH * NB)
    )

    in_maps = []
    for c in range(NCORES):
        sl = slice(c * ROWS, (c + 1) * ROWS)
        # vnT[p, m*KCH*128 + k2*256 + i*128 + r] = Vn8_slab[m*128+r, (2k2+i)*128+p]
        vnT = np.ascontiguousarray(
            Vn8[sl]
            .reshape(MCH, 128, KD2, 2, 128)
            .transpose(4, 0, 2, 3, 1)
            .reshape(128, KCH * ROWS)
        )
        in_maps.append({"vnT": vnT, "anT": anT})
    return in_maps, diag, L_pre


def _assemble(outs, diag, L_pre):
    """O(N) final reduction on host, f64."""
    def _core_rowsum(c):
        r = outs[c]["rowsum"].astype(np.float64)  # [128, 17]
        r = np.concatenate([r[:, :15], (r[:, 15] + r[:, 16])[:, None]], axis=1)
        return r.reshape(128, NCH // 2, MCH).sum(1).T.reshape(ROWS)

    rowsum = np.concatenate([_core_rowsum(c) for c in range(NCORES)])
    colsum = np.zeros(N, dtype=np.float64)
    for c in range(NCORES):
        colsum += outs[c]["colsum"].astype(np.float64).reshape(N)

    dE = np.exp(diag)
    pos = np.exp(diag - MARGIN)
    neg_V = rowsum - dE
    neg_A = colsum - dE
    L_V = np.log(pos / (pos + neg_V)).sum()
    L_A = np.log(pos / (pos + neg_A)).sum()

    loss = BALANCE * (-1.0 / BIAS) * (L_V + L_A) + (1.0 - BALANCE) * L_pre
    return np.array(loss, dtype=np.float32)


def kernel(pre_VF, pre_AF, back_VF, back_AF):
    global LAST_RESULT
    from concourse import bass_utils

    nc = _get_nc()
    in_maps, diag, L_pre = _prep_inputs(pre_VF, pre_AF, back_VF, back_AF)
    res = bass_utils.run_bass_kernel_spmd(nc, in_maps, core_ids=list(range(NCORES)))
    LAST_RESULT = res
    return _assemble(res.results, diag, L_pre)
